# revision 6
# baseline (speedup 1.0000x reference)
"""Trainium2 Bass kernel for nn_BaselineBlock_SCA_Modulated (optimized v2).

Sharding: 8 cores = 2 batch x 4 D-slabs of 16 planes. Conv pass uses fp8
DoubleRow matmuls (4 taps per instruction at 0.5 cyc/col); LayerNorm mean
subtraction is folded into extra matmul rows multiplying a flattened
mean*rstd field; rsqrt is computed with a Newton iteration on DVE so the
Act engine only ever loads the gelu table set.
"""
import numpy as np
import ml_dtypes

C, DW, SD = 64, 128, 512
D, H, W = 64, 64, 64
NPL = 16
NHALO = NPL + 2
PW = 66
PSZ = PW * PW + 2          # 4358
HWC = H * W                # 4096
EPS_V = 1e-7
bf = ml_dtypes.bfloat16
f8 = ml_dtypes.float8_e4m3

# rsqrt seed: a/v + b + c*v, then 2 Newton iterations (fit on [0.08, 10])
RS_A, RS_B, RS_C = 0.27183101683991684, 0.679839007976355, -0.043888329744271516

_CACHE = {}


def _ins_dim(ap_obj, pos, stride, count):
    from concourse.ap import AP
    lst = [list(x) for x in ap_obj.ap]
    lst = lst[:pos] + [[int(stride), int(count)]] + lst[pos:]
    return AP(ap_obj.tensor, ap_obj.offset, lst)


def _build():
    import concourse.bacc as bacc
    import concourse.mybir as mybir
    import concourse.tile as tile
    from concourse.mybir import ActivationFunctionType as AF, AluOpType as ALU
    from concourse.ap import AP as _AP

    BF = mybir.dt.bfloat16
    F32 = mybir.dt.float32
    F8 = mybir.dt.float8e4
    AX = mybir.AxisListType
    DR = mybir.MatmulPerfMode.DoubleRow

    nc = bacc.Bacc("TRN2", target_bir_lowering=False, debug=False, num_devices=8)

    dram = {}
    def din(name, shape, dt):
        dram[name] = nc.dram_tensor(name, shape, dt, kind="ExternalInput")
        return dram[name]

    din("inp_t", [NHALO, C, HWC], BF)
    din("inp_f", [NPL, C, HWC], BF)
    din("wdrF", [128, 7, 2, 128], F8)
    din("wdrS", [128, 3, 2, 128], F8)
    din("wsc", [128, NPL, 2, 128], F8)
    din("sw3v", [64, 1], F32)
    din("ind_pad", [9, PSZ], F8)
    din("fmr", [4, PSZ], F8)
    din("modb", [128, 1], F32)
    din("sdp", [128, 1], F32)
    din("w3Tc", [128, 64], BF)
    din("scawT", [128, 128], BF)
    din("scab", [128, 1], F32)
    din("b3b", [64, 1], F32)
    din("w4e", [65, 128], BF)
    din("b4", [128, 1], F32)
    din("w5g", [128, 64], BF)
    din("i128", [128, 128], BF)
    din("i128f", [128, 128], F32)
    din("i64", [64, 64], BF)
    din("i64s", [64, 64], BF)
    out_d = nc.dram_tensor("out", [NPL, C, HWC], F32, kind="ExternalOutput")

    xg_scr = nc.dram_tensor("xg_scr", [NPL, 128, HWC], F8)
    cc_a = nc.dram_tensor("cc_a", [128, 1], F32)
    cc_b = nc.dram_tensor("cc_b", [128, 1], F32)

    from contextlib import ExitStack
    with tile.TileContext(nc) as tc, ExitStack() as stk:
        cpool = stk.enter_context(tc.tile_pool(name="const", bufs=1))
        rpool = stk.enter_context(tc.tile_pool(name="ring", bufs=1))
        wpool = stk.enter_context(tc.tile_pool(name="work", bufs=2))
        p2p = stk.enter_context(tc.tile_pool(name="p2", bufs=2))
        psA = stk.enter_context(tc.tile_pool(name="psA", bufs=2, space="PSUM"))
        psB = stk.enter_context(tc.tile_pool(name="psB", bufs=2, space="PSUM"))

        def const(name, shape, dt):
            t = cpool.tile(shape, dt, tag=name, name=name)
            nc.sync.dma_start(t[:], dram[name][:])
            return t

        wdrF = const("wdrF", [128, 7, 2, 128], F8)
        wdrS = const("wdrS", [128, 3, 2, 128], F8)
        wsc = const("wsc", [128, NPL, 2, 128], F8)
        sw3v = const("sw3v", [64, 1], F32)
        modb = const("modb", [128, 1], F32)
        sdp = const("sdp", [128, 1], F32)
        w3Tc = const("w3Tc", [128, 64], BF)
        scawT = const("scawT", [128, 128], BF)
        scab = const("scab", [128, 1], F32)
        b3b = const("b3b", [64, 1], F32)
        w4e = const("w4e", [65, 128], BF)
        b4 = const("b4", [128, 1], F32)
        w5g = const("w5g", [128, 64], BF)
        i128 = const("i128", [128, 128], BF)
        i128f = const("i128f", [128, 128], F32)
        i64 = const("i64", [64, 64], BF)
        i64s = const("i64s", [64, 64], BF)

        pools = cpool.tile([128, NPL * 8], F32, tag="pools")
        w3Tp = cpool.tile([128, 64], F8, tag="w3Tp")

        # mega tile: 5 t1 slots | 5 th slots | 2 ind+field areas  (fp8)
        NRING = 4
        NSL = 2 * NRING + 2
        mega = rpool.tile([128, NSL * PSZ], F8, tag="mega", name="mega")
        fmr = dram["fmr"]  # field ring in DRAM (uploaded zeroed)

        T1B = lambda s: s * PSZ
        THB = lambda s: (NRING + s) * PSZ
        INDB = lambda a: (2 * NRING + a) * PSZ

        # ---- one-time zero init (pads + ind/field area) ----
        nc.gpsimd.memset(mega[:, INDB(0):INDB(0) + 2 * PSZ], 0.0)
        for s in range(NRING):
            for base, lo in ((T1B(s), True), (THB(s), True)):
                v = mega[0:64, base:base + PW * PW].rearrange(
                    "p (r w) -> p r w", w=PW)
                vu = mega[64:128, base:base + PW * PW].rearrange(
                    "p (r w) -> p r w", w=PW)
                nc.vector.memset(v[:, :, 0:1], 0.0)
                nc.vector.memset(v[:, :, 65:66], 0.0)
                nc.vector.memset(v[:, 0:1, :], 0.0)
                nc.vector.memset(v[:, 63:66, :], 0.0)
                nc.vector.memset(vu[:, :, 63:66], 0.0)
                nc.vector.memset(vu[:, 0:1, :], 0.0)
                nc.vector.memset(vu[:, 65:66, :], 0.0)
                nc.vector.memset(mega[:, base + PW * PW:base + PSZ], 0.0)
        # indicator patterns into both areas (rows 0-8)
        for a in range(2):
            nc.sync.dma_start(mega[0:9, INDB(a):INDB(a) + PSZ], dram["ind_pad"][:])

        # ---------------- PASS 1 ----------------
        def ln1_a(p):
            xT = wpool.tile([128, 32, 64], BF, tag="xT")
            nc.sync.dma_start_transpose(xT[:], dram["inp_t"][p])
            # stats: folds + reduces
            sq = wpool.tile([128, 32, 64], BF, tag="sq", bufs=1)
            nc.vector.tensor_mul(sq[:], xT[:], xT[:])
            xf1 = wpool.tile([128, 32, 32], BF, tag="xf1")
            nc.vector.tensor_add(xf1[:], xT[:, :, 0:32], xT[:, :, 32:64])
            xf2 = wpool.tile([128, 32, 16], BF, tag="xf2")
            nc.vector.tensor_add(xf2[:], xf1[:, :, 0:16], xf1[:, :, 16:32])
            msum = wpool.tile([128, 32], F32, tag="msum")
            nc.vector.tensor_reduce(msum[:], xf2[:], axis=AX.X, op=ALU.add)
            sf1 = wpool.tile([128, 32, 32], BF, tag="sf1")
            nc.vector.tensor_add(sf1[:], sq[:, :, 0:32], sq[:, :, 32:64])
            sf2 = wpool.tile([128, 32, 16], BF, tag="sf2")
            nc.vector.tensor_add(sf2[:], sf1[:, :, 0:16], sf1[:, :, 16:32])
            qsum = wpool.tile([128, 32], F32, tag="qsum")
            nc.vector.tensor_reduce(qsum[:], sf2[:], axis=AX.X, op=ALU.add)
            rv, mrv = _rsqrt_chain(msum, qsum, wpool, "")
            # rho-mul (DVE; quantization to fp8 happens at transpose evac)
            xq = wpool.tile([128, 32, 64], BF, tag="xq", bufs=3)
            for hh in range(2):
                cs = slice(16 * hh, 16 * (hh + 1))
                nc.vector.tensor_mul(
                    xq[:, cs, :], xT[:, cs, :],
                    rv[:, cs].unsqueeze(2).broadcast_to([128, 16, 64]))
            return xq, mrv

        def ln1_b(p, xq, mrv):
            s = p % NRING
            # transposes + evac into t1 slot (Act groups 0-1, DVE groups 2-3)
            base = T1B(s)
            for g4 in range(4):
                psT = psA.tile([64, 1024], BF, tag="trX", bufs=3)
                for k in range(8):
                    nc.tensor.transpose(
                        psT[:, k * 128:(k + 1) * 128], xq[:, 8 * g4 + k, :],
                        i128[:])
                dst = mega[0:64, base:base + PW * PW].rearrange(
                    "p (r w) -> p r w", w=PW)[:, 1 + 16 * g4:17 + 16 * g4, 1:65]
                src = psT[:].rearrange("p (r w) -> p r w", w=64)
                if g4 < 3:
                    nc.scalar.activation(dst, src, AF.Copy)
                else:
                    nc.vector.tensor_copy(dst, src)
            # shift copies (fp8): t1 upper (+2 col), th lower (+2 row), th plain
            thb = THB(s)
            nc.sync.dma_start(mega[64:128, base:base + PSZ - 2],
                              mega[0:64, base + 2:base + PSZ])
            nc.sync.dma_start(mega[0:64, thb:thb + PSZ - 132],
                              mega[0:64, base + 132:base + PSZ])
            nc.sync.dma_start(mega[64:128, thb:thb + PSZ],
                              mega[0:64, base:base + PSZ])
            # field row: mrv -> [32, 128] -> fp8 -> padded flat row in fmr
            psF = psA.tile([32, 128], F32, tag="trX", bufs=3)
            nc.tensor.transpose(psF[:], mrv[:], i128f[:])
            fT = wpool.tile([32, 128], F8, tag="fT")
            nc.vector.tensor_copy(fT[:], psF[:])
            dstf = fmr[s:s + 1, 0:PW * PW].rearrange(
                "p (r w) -> p r w", w=PW)[:, 1:65, 1:65]
            nc.sync.dma_start(dstf, fT[:])

        def conv_plane(d):
            # assemble 27 shifted field rows into ind area (rows 9..35)
            a = d % 2
            indb = INDB(a)
            for kd in range(3):
                sfm = (d + kd) % NRING
                base = fmr[sfm:sfm + 1, 0:4224]
                src = _AP(base.tensor, base.offset,
                          [[PW, 3], [1, 3], [1, 4224]])
                nc.sync.dma_start(
                    mega[9 + 9 * kd:18 + 9 * kd, indb + 67:indb + 67 + 4224],
                    src)
            slot = [(d + kd) % NRING for kd in range(3)]
            xgp = wpool.tile([128, HWC], F8, tag="xgp")
            for cb in range(8):
                ps = psB.tile([128, 512], F32, tag="mmA", bufs=3)
                # window offsets
                def Pw(kd, kh):
                    return T1B(slot[kd]) + (8 * cb + kh) * PW
                def Tw(kd):
                    return THB(slot[kd]) + (8 * cb) * PW + 1
                def Sw(kd):
                    return THB(slot[kd]) + (8 * cb + 1) * PW + 1
                CFw = indb + (8 * cb + 1) * PW + 1
                pairs = [
                    (Pw(0, 0), Pw(0, 1), 0), (Pw(0, 2), Pw(1, 0), 1),
                    (Pw(1, 1), Pw(1, 2), 2), (Pw(2, 0), Pw(2, 1), 3),
                    (Pw(2, 2), Tw(0), 4), (Tw(1), Tw(2), 5),
                    (Sw(0), Sw(2), 6),
                ]
                SWI = {1: 0, 5: 1, 6: 2}
                for oA, oB, i in pairs:
                    if oB >= oA:
                        w_ap = wdrF[:, i]
                    else:
                        oA, oB = oB, oA
                        w_ap = wdrS[:, SWI[i]]
                    tmpl = mega[0:128, oA:oA + 528].rearrange(
                        "p (h w) -> p h w", w=PW)[:, 0:8, 0:64]
                    rhs = _ins_dim(tmpl, 1, oB - oA, 2)
                    nc.tensor.matmul(ps[:], w_ap, rhs, start=(i == 0),
                                     stop=False, perf_mode=DR)
                # DR8: S1 + corr/field slice
                oA, oB = Sw(1), CFw
                tmpl = mega[0:128, oA:oA + 528].rearrange(
                    "p (h w) -> p h w", w=PW)[:, 0:8, 0:64]
                rhs = _ins_dim(tmpl, 1, oB - oA, 2)
                nc.tensor.matmul(ps[:], wsc[:, d], rhs, start=False,
                                 stop=True, perf_mode=DR)
                nc.scalar.activation(
                    xgp[:, cb * 512:(cb + 1) * 512], ps[:], AF.Gelu,
                    bias=modb[:], scale=sdp[:],
                    accum_out=pools[:, d * 8 + cb:d * 8 + cb + 1])
            nc.sync.dma_start(xg_scr[d], xgp[:])

        def _rsqrt_chain(msum, qsum, pool, sfx):
            def t32(tag):
                return pool.tile([128, 32], F32, tag=tag + sfx,
                                 name=tag + sfx)
            qv = t32("qv")
            nc.vector.tensor_scalar(qv[:], qsum[:], 1.0 / 63.0, EPS_V,
                                    op0=ALU.mult, op1=ALU.add)
            t1m = t32("t1m")
            nc.vector.tensor_mul(t1m[:], msum[:], msum[:])
            var = t32("var")
            nc.vector.scalar_tensor_tensor(var[:], t1m[:], -1.0 / 4032.0,
                                           qv[:], op0=ALU.mult, op1=ALU.add)
            r1 = t32("r1")
            nc.vector.reciprocal(r1[:], var[:])
            vb = t32("vb")
            nc.vector.tensor_scalar(vb[:], var[:], RS_C, RS_B,
                                    op0=ALU.mult, op1=ALU.add)
            y0 = t32("y0")
            nc.vector.scalar_tensor_tensor(y0[:], r1[:], RS_A, vb[:],
                                           op0=ALU.mult, op1=ALU.add)
            yc = y0
            for it in range(1):
                t = t32(f"t{it}")
                nc.vector.tensor_mul(t[:], yc[:], yc[:])
                u = t32(f"u{it}")
                nc.vector.tensor_mul(u[:], t[:], var[:])
                w = t32(f"w{it}")
                nc.vector.tensor_scalar(w[:], u[:], -0.5, 1.5,
                                        op0=ALU.mult, op1=ALU.add)
                yn = t32(f"yn{it}")
                nc.vector.tensor_mul(yn[:], yc[:], w[:])
                yc = yn
            mrv = t32("mrv")
            nc.vector.scalar_tensor_tensor(mrv[:], msum[:], 1.0 / 64.0,
                                           yc[:], op0=ALU.mult, op1=ALU.mult)
            return yc, mrv

        stA = {}
        for i in range(NHALO + 2):
            if i < NHALO:
                stA[i] = ln1_a(i)
            if 1 <= i and i - 1 < NHALO:
                ln1_b(i - 1, *stA.pop(i - 1))
            if i >= 4:
                conv_plane(i - 4)

        # ---------------- pooled -> gate -> w3Tp ----------------
        pooled = cpool.tile([128, 1], F32, tag="pooled")
        nc.vector.tensor_reduce(pooled[:], pools[:], axis=AX.X, op=ALU.add)
        nc.sync.dma_start(cc_a[:], pooled[:])
        nc.gpsimd.collective_compute(
            "AllReduce", ALU.add,
            replica_groups=[[0, 1, 2, 3], [4, 5, 6, 7]],
            ins=[cc_a[:]], outs=[cc_b[:]])
        pooled2f = cpool.tile([128, 1], F32, tag="pooled2f", name="pooled2f")
        nc.sync.dma_start(pooled2f[:], cc_b[:])
        pooled2 = cpool.tile([128, 1], BF, tag="pooled2", name="pooled2")
        nc.vector.tensor_copy(pooled2[:], pooled2f[:])
        psg = psB.tile([128, 1], F32, tag="ps35", bufs=2)
        nc.tensor.matmul(psg[:], scawT[:], pooled2[:], start=True, stop=True)
        gate = cpool.tile([128, 1], F32, tag="gatev")
        nc.scalar.activation(gate[:], psg[:], AF.Identity, bias=scab[:])
        nc.vector.tensor_scalar_mul(w3Tp[:], w3Tc[:], gate[:])

        # ---------------- PASS 2 (3-stage software pipeline) ----------------
        def p2_s1(d):
            xgt = p2p.tile([128, HWC], F8, tag="xgt")
            nc.sync.dma_start(xgt[:], xg_scr[d])
            inb = p2p.tile([64, HWC], BF, tag="inb")
            nc.sync.dma_start(inb[:], dram["inp_f"][d])
            y = p2p.tile([64, HWC], BF, tag="y", bufs=3)
            for cb in range(8):
                sl = slice(cb * 512, (cb + 1) * 512)
                ps3 = psB.tile([64, 512], F32, tag="ps35", bufs=2)
                nc.tensor.matmul(ps3[:], w3Tp[:], xgt[:, sl], start=True,
                                 stop=False)
                nc.tensor.matmul(ps3[:], i64s[:], inb[:, sl], start=False,
                                 stop=True)
                if cb % 2 == 0 or cb == 7:
                    nc.scalar.activation(y[:, sl], ps3[:], AF.Identity,
                                         bias=b3b[:], scale=sw3v[:])
                else:
                    nc.vector.tensor_scalar(y[:, sl], ps3[:], sw3v[:],
                                            b3b[:], op0=ALU.mult, op1=ALU.add)
            return y

        def p2_s2(d, y):
            # yT transposes -> sbuf copy -> per-group stats (group-0 stats
            # run on DVE while PE transposes group 1)
            yTs = wpool.tile([128, 32, 64], BF, tag="yTs", bufs=1)
            sq2 = wpool.tile([128, 32, 64], BF, tag="sq2", bufs=1)
            yf1 = wpool.tile([128, 32, 32], BF, tag="yf1", bufs=1)
            yf2 = wpool.tile([128, 32, 16], BF, tag="yf2")
            msum2 = wpool.tile([128, 32], F32, tag="msum2")
            sf1b = wpool.tile([128, 32, 32], BF, tag="sf1b", bufs=1)
            sf2b = wpool.tile([128, 32, 16], BF, tag="sf2b")
            qsum2 = wpool.tile([128, 32], F32, tag="qsum2")
            for g in range(2):
                cs = slice(16 * g, 16 * (g + 1))
                yp = psA.tile([128, 1024], BF, tag="trX", name=f"yTp{g}", bufs=3)
                for k in range(16):
                    cg = 16 * g + k
                    nc.tensor.transpose(
                        yp[:, k * 64:(k + 1) * 64],
                        y[:, cg * 128:(cg + 1) * 128], i64[:])
                nc.vector.tensor_copy(
                    yTs[:, cs, :],
                    yp[:].rearrange("p (c k) -> p c k", k=64))
                nc.vector.tensor_mul(sq2[:, cs, :], yTs[:, cs, :],
                                     yTs[:, cs, :])
                nc.vector.tensor_add(yf1[:, cs, :], yTs[:, cs, 0:32],
                                     yTs[:, cs, 32:64])
                nc.vector.tensor_add(yf2[:, cs, :], yf1[:, cs, 0:16],
                                     yf1[:, cs, 16:32])
                nc.vector.tensor_reduce(msum2[:, cs], yf2[:, cs, :],
                                        axis=AX.X, op=ALU.add)
                nc.vector.tensor_add(sf1b[:, cs, :], sq2[:, cs, 0:32],
                                     sq2[:, cs, 32:64])
                nc.vector.tensor_add(sf2b[:, cs, :], sf1b[:, cs, 0:16],
                                     sf1b[:, cs, 16:32])
                nc.vector.tensor_reduce(qsum2[:, cs], sf2b[:, cs, :],
                                        axis=AX.X, op=ALU.add)
            rv2, mrv2 = _rsqrt_chain(msum2, qsum2, wpool, "b")
            yr = p2p.tile([65, HWC], BF, tag="yr")
            xl2 = wpool.tile([128, 32, 64], BF, tag="xl2", bufs=1)
            for hh in range(2):
                cs = slice(16 * hh, 16 * (hh + 1))
                nc.vector.tensor_mul(
                    xl2[:, cs, :], yTs[:, cs, :],
                    rv2[:, cs].unsqueeze(2).broadcast_to([128, 16, 64]))
            psM = psA.tile([32, 128], F32, tag="trX", bufs=3)
            nc.tensor.transpose(psM[:], mrv2[:], i128f[:])
            mrfT = wpool.tile([32, 128], BF, tag="mrfT")
            nc.vector.tensor_copy(mrfT[:], psM[:])
            nc.sync.dma_start(yr[64:65, 0:HWC], mrfT[:])
            for g4 in range(4):
                psX = psA.tile([64, 1024], BF, tag="trX", bufs=3)
                for k in range(8):
                    nc.tensor.transpose(
                        psX[:, k * 128:(k + 1) * 128],
                        xl2[:, 8 * g4 + k, :], i128[:])
                if g4 % 2 == 0:
                    nc.scalar.activation(
                        yr[0:64, 1024 * g4:1024 * (g4 + 1)], psX[:], AF.Copy)
                else:
                    nc.vector.tensor_copy(
                        yr[0:64, 1024 * g4:1024 * (g4 + 1)], psX[:])
            return yr

        def p2_s3(d, y, yr):
            for hf in range(4):
                outp = p2p.tile([64, HWC // 4], F32, tag="outp")
                for cq in range(2):
                    cb = 2 * hf + cq
                    sl = slice(cb * 512, (cb + 1) * 512)
                    lsl = slice(cq * 512, (cq + 1) * 512)
                    ps4 = psB.tile([128, 512], F32, tag="mmA", bufs=3)
                    nc.tensor.matmul(ps4[:], w4e[:], yr[0:65, sl], start=True,
                                     stop=True)
                    xg2 = wpool.tile([128, 512], BF, tag="xg2")
                    nc.scalar.activation(xg2[:], ps4[:], AF.Gelu, bias=b4[:])
                    ps5 = psB.tile([64, 512], F32, tag="ps35", bufs=2)
                    nc.tensor.matmul(ps5[:], w5g[:], xg2[:], start=True,
                                     stop=False)
                    nc.tensor.matmul(ps5[:], i64[:], y[:, sl], start=False,
                                     stop=True)
                    if cb % 4 == 3:
                        nc.vector.tensor_copy(outp[:, lsl], ps5[:])
                    else:
                        nc.scalar.activation(outp[:, lsl], ps5[:], AF.Copy)
                nc.sync.dma_start(
                    out_d[d][:, hf * 1024:(hf + 1) * 1024], outp[:])

        s1 = {}
        s2 = {}
        for i in range(NPL + 2):
            if i < NPL:
                s1[i] = p2_s1(i)
            if i >= 1 and i - 1 < NPL:
                s2[i - 1] = p2_s2(i - 1, s1[i - 1])
            if i >= 2:
                p2_s3(i - 2, s1.pop(i - 2), s2.pop(i - 2))

    nc.compile()
    return nc


def _f8(x):
    return np.clip(x, -224.0, 224.0).astype(f8)


def _host_prep(inputs):
    inp = np.asarray(inputs["inp"], np.float32)
    style = np.asarray(inputs["style_vector"], np.float32)
    w1 = np.asarray(inputs["w1"], np.float32)
    b1 = np.asarray(inputs["b1"], np.float32)
    mod_w = np.asarray(inputs["mod_w"], np.float32)
    mod_b = np.asarray(inputs["mod_b"], np.float32)
    style_w = np.asarray(inputs["style_w"], np.float32)
    style_b = np.asarray(inputs["style_b"], np.float32)
    sca_w = np.asarray(inputs["sca_w"], np.float32)
    sca_b = np.asarray(inputs["sca_b"], np.float32)
    w3 = np.asarray(inputs["w3"], np.float32)
    b3 = np.asarray(inputs["b3"], np.float32)
    w4 = np.asarray(inputs["w4"], np.float32)
    b4 = np.asarray(inputs["b4"], np.float32)
    w5 = np.asarray(inputs["w5"], np.float32)
    b5 = np.asarray(inputs["b5"], np.float32)
    ln1_w = np.asarray(inputs["ln1_w"], np.float32).reshape(C)
    ln2_w = np.asarray(inputs["ln2_w"], np.float32).reshape(C)
    beta = np.asarray(inputs["beta"], np.float32).reshape(C)
    gamma = np.asarray(inputs["gamma"], np.float32).reshape(C)

    s = style @ style_w.T + style_b
    k2 = (mod_w ** 2).sum(axis=(1, 2, 3, 4))
    demod = 1.0 / np.sqrt(k2[None] * s * s + 1e-8)
    sdv = s * demod                                     # [B, DW]

    W1t = w1 * ln1_w[None, :]                           # [DW, C]
    wdw = mod_w[:, 0]                                   # [DW, 3, 3, 3]
    W1sum = W1t.sum(axis=1)                             # [DW]

    # conv weight scale SW (power of two, target max ~100)
    def B_(kd, kh, kw):
        return (W1t * wdw[:, kd, kh, kw][:, None]).T    # [C, DW]

    maxw = max(np.abs(B_(kd, kh, kw)).max()
               for kd in range(3) for kh in range(3) for kw in range(3))
    # corr / field coefficient magnitudes share the same fp8 scale
    W1sum_pre = W1t.sum(axis=1)
    maxf = max(np.abs(wdw[:, kd, kh, kw] * W1sum_pre).max()
               for kd in range(3) for kh in range(3) for kw in range(3))
    maxc = np.abs(mod_w[:, 0].sum(axis=(1, 2))).max() * 9 * np.abs(b1).max()
    SW = 2.0 ** np.floor(np.log2(100.0 / max(maxw, maxf, maxc / 4)))

    # per-DR lhsT [128, 128] blocks (K rows x M cols), fp8 scaled by SW
    def blk(lo, hi):
        m = np.zeros((128, 128), np.float32)
        if lo is not None:
            m[0:64] = lo
        if hi is not None:
            m[64:128] = hi
        return m * SW

    Wp = {}
    for kd in range(3):
        for kh in range(3):
            Wp[("P", kd, kh)] = blk(B_(kd, kh, 0), B_(kd, kh, 2))
        Wp[("T", kd)] = blk(B_(kd, 2, 1), B_(kd, 0, 1))
        Wp[("S", kd)] = blk(None, B_(kd, 1, 1))

    DRK = [(("P", 0, 0), ("P", 0, 1)), (("P", 0, 2), ("P", 1, 0)),
           (("P", 1, 1), ("P", 1, 2)), (("P", 2, 0), ("P", 2, 1)),
           (("P", 2, 2), ("T", 0)), (("T", 1), ("T", 2)),
           (("S", 0), ("S", 2))]
    wdrF = np.zeros((128, 7, 2, 128), np.float32)
    wdrS = np.zeros((128, 3, 2, 128), np.float32)
    SWI = {1: 0, 5: 1, 6: 2}
    for i, (ka, kb) in enumerate(DRK):
        wdrF[:, i, 0] = Wp[ka]
        wdrF[:, i, 1] = Wp[kb]
        if i in SWI:
            wdrS[:, SWI[i], 0] = Wp[kb]
            wdrS[:, SWI[i], 1] = Wp[ka]

    # b1 correction + mean-field coefficients (slice B of DR8)
    def S_(cd, ch, cw):
        vd = {0: [1, 2], 1: [0, 1, 2], 2: [0, 1]}[cd]
        vh = {0: [1, 2], 1: [0, 1, 2], 2: [0, 1]}[ch]
        vw = {0: [1, 2], 1: [0, 1, 2], 2: [0, 1]}[cw]
        return wdw[:, vd][:, :, vh][:, :, :, vw].sum(axis=(1, 2, 3))

    def corr_for(dcase):
        c = np.zeros((9, 128), np.float32)
        base = S_(dcase, 1, 1)
        c[0] = base
        c[1] = S_(dcase, 0, 1) - base
        c[2] = S_(dcase, 2, 1) - base
        c[3] = S_(dcase, 1, 0) - base
        c[4] = S_(dcase, 1, 2) - base
        c[5] = S_(dcase, 0, 0) - S_(dcase, 0, 1) - S_(dcase, 1, 0) + base
        c[6] = S_(dcase, 0, 2) - S_(dcase, 0, 1) - S_(dcase, 1, 2) + base
        c[7] = S_(dcase, 2, 0) - S_(dcase, 2, 1) - S_(dcase, 1, 0) + base
        c[8] = S_(dcase, 2, 2) - S_(dcase, 2, 1) - S_(dcase, 1, 2) + base
        return c * b1[None, :]

    corr_tab = {c: corr_for(c) for c in (0, 1, 2)}

    g = np.zeros((9, 64, 64), np.float32)
    g[0] = 1.0
    g[1, 0, :] = 1.0
    g[2, 63, :] = 1.0
    g[3, :, 0] = 1.0
    g[4, :, 63] = 1.0
    g[5, 0, 0] = 1.0
    g[6, 0, 63] = 1.0
    g[7, 63, 0] = 1.0
    g[8, 63, 63] = 1.0
    ind_pad = np.zeros((9, PSZ), np.float32)
    ipv = ind_pad[:, 0:PW * PW].reshape(9, PW, PW)
    ipv[:, 1:65, 1:65] = g

    # field coefficients: row 9 + 9*kd + 3*kh + kw = -wdw[:,kd,kh,kw]*W1sum
    fcoef = np.zeros((27, 128), np.float32)
    for kd in range(3):
        for kh in range(3):
            for kw in range(3):
                fcoef[9 * kd + 3 * kh + kw] = -wdw[:, kd, kh, kw] * W1sum

    # w3 scale
    w3b = w3.T * beta[None, :]                          # [DW, C] cols=out c
    SW3 = 2.0 ** np.floor(np.log2(15.0 / np.abs(w3b).max()))

    common = dict(
        wdrF=_f8(wdrF), wdrS=_f8(wdrS),
        ind_pad=_f8(ind_pad),
        modb=mod_b.reshape(128, 1).astype(np.float32),
        w3Tc=(w3b * SW3).astype(bf),
        scawT=(sca_w.T / float(D * H * W)).astype(bf),
        scab=sca_b.reshape(128, 1).astype(np.float32),
        b3b=(beta * b3 + gamma * b5).reshape(64, 1).astype(np.float32),
        w4e=np.concatenate(
            [(w4 * ln2_w[None, :]).T,
             -(w4 * ln2_w[None, :]).sum(axis=1)[None, :]], 0).astype(bf),
        b4=b4.reshape(128, 1).astype(np.float32),
        w5g=(w5.T * gamma[None, :]).astype(bf),
        i128=np.eye(128, dtype=np.float32).astype(bf),
        i128f=np.eye(128, dtype=np.float32),
        i64=np.eye(64, dtype=np.float32).astype(bf),
        i64s=(np.eye(64, dtype=np.float32) * SW3).astype(bf),
        sw3v=np.full((64, 1), 1.0 / SW3, np.float32),
        fmr=np.zeros((4, PSZ), f8),
    )

    in_maps = []
    for k in range(8):
        b_, d0 = k // 4, (k % 4) * NPL
        ip = inp[b_]
        halo = np.zeros((NHALO, C, HWC), np.float32)
        lo, hi = max(d0 - 1, 0), min(d0 + NPL + 1, D)
        halo[lo - (d0 - 1):hi - (d0 - 1)] = (
            ip[:, lo:hi].transpose(1, 0, 2, 3).reshape(hi - lo, C, HWC))
        wsc = np.zeros((128, NPL, 2, 128), np.float32)
        for i in range(NPL):
            dg = d0 + i
            dcase = 0 if dg == 0 else (2 if dg == D - 1 else 1)
            wsc[:, i, 0] = Wp[("S", 1)]
            wsc[0:9, i, 1] = corr_tab[dcase] * SW
            wsc[9:36, i, 1] = fcoef * SW
        m = dict(common)
        m["inp_t"] = halo.astype(bf)
        m["inp_f"] = np.ascontiguousarray(
            ip[:, d0:d0 + NPL].transpose(1, 0, 2, 3).reshape(
                NPL, C, HWC)).astype(bf)
        m["wsc"] = _f8(wsc)
        m["sdp"] = (sdv[b_] / SW).reshape(128, 1).astype(np.float32)
        in_maps.append(m)
    return in_maps


def kernel(**inputs):
    from concourse.bass_utils import run_bass_kernel_spmd
    in_maps = _host_prep(inputs)
    if "nc" not in _CACHE:
        _CACHE["nc"] = _build()
    nc = _CACHE["nc"]
    res = run_bass_kernel_spmd(nc, in_maps, list(range(8)))
    _CACHE["last_res"] = res
    out = np.empty((2, C, D, H, W), np.float32)
    for k in range(8):
        b_, d0 = k // 4, (k % 4) * NPL
        o = res.results[k]["out"]
        out[b_, :, d0:d0 + NPL] = o.reshape(NPL, C, H, W).transpose(1, 0, 2, 3)
    return out
